# revision 23
# baseline (speedup 1.0000x reference)
"""Trainium2 Bass kernel for nn_Conv_27693949125154.

Each 128-dim vector is a 16x8 image; valid 3x3 conv with the fixed kernel
[[1,0,1],[0,1,0],[1,0,1]] then relu -> 84 outputs (14x6).

The conv kernel decomposes as outer([1,0,1],[1,0,1]) + center tap:
    h(i,j) = x(i,j) + x(i,j+2)            (horizontal, 16x6)
    out(i,j) = relu(h(i,j) + h(i+2,j) + x(i+1,j+1))   (14x6)

Layout: letters (B*W rows) on SBUF partitions, pixels along the free dim.
All stencil taps are free-dim strided slices.

Measured on HW: f32 DVE tensor-adds run 1 elem/cycle and stretch badly
under SBUF contention; bf16 adds hit the 2x DVE mode (0.52 ns/elem) with
no stretch; Pool dtype-converting ops are ~3.5 ns/elem (avoid). So: cast
each chunk f32->bf16 once on ACT, keep every DVE op bf16.

  whole input f32 -> resident SBUF tile, reads issued upfront on both
  HWDGE rings (deep read-ahead; input fully lands by ~45us).
  per 28-letter chunk:
    cast  (ACT): xbc = bf16(xt[chunk])     128/letter
    op1   (DVE): r = xbc(i,j)+xbc(i,j+2)   16x6/letter, 2x mode
    op2   (DVE): s = r(0:14)+r(2:16)       14x6
    op3   (DVE): u = s + xbc(i+1,j+1)      14x6
    relu  (ACT): ot = relu(u)              bf16 -> f32
    store (scalar ring)

Pure data parallel over 8 NeuronCores (batch sharding, no comm).
"""

import numpy as np

import concourse.bass as bass
import concourse.mybir as mybir
from concourse import tile
from concourse.bass_utils import run_bass_kernel_spmd

# Full problem: x (16384, 14, 128) f32 -> out (16384, 14, 84) f32
B, W, L = 16384, 14, 128
OUT = 84
N_CORES = 8
ROWS = B * W                     # 229376 letters total
ROWS_PER_CORE = ROWS // N_CORES  # 28672
P = 128                          # SBUF partitions

F32 = mybir.dt.float32
BF16 = mybir.dt.bfloat16


def split_multi_waits(nc, max_waits=1):
    """walrus CoreV3 codegen rejects instructions with several sync-wait
    conditions; hoist extras onto NOPs inserted just before, same engine."""
    for f in nc.m.functions:
        for blk in f.blocks:
            new = []
            for inst in blk.instructions:
                si = inst.sync_info
                if si is not None and si.on_wait and len(si.on_wait) > max_waits:
                    waits = list(si.on_wait)
                    head, tail = waits[:-max_waits], waits[-max_waits:]
                    for k, w in enumerate(head):
                        new.append(
                            mybir.InstNoOp(
                                name=f"{inst.name}-wsplit{k}",
                                engine=inst.engine,
                                ins=[],
                                outs=[],
                                sync_info=mybir.SyncInfo(on_wait=[w], on_update=[]),
                            )
                        )
                    inst.sync_info = mybir.SyncInfo(
                        on_wait=tail, on_update=list(si.on_update)
                    )
                new.append(inst)
            blk.instructions = new


def build_program(rows=ROWS_PER_CORE, read_sizes=None, chunk_sizes=None,
                  split_waits=True, xb_bufs=2, work_bufs=2, u_bufs=3, o_bufs=4,
                  pool_relu_chunks=(), dve_cast_chunks=(), pool_op2_min=99,
                  act_cast_letters=99):
    """Per-core program: x [rows,128] f32 -> y [rows,84] f32.

    Reads are issued upfront as independent slice-DMAs into a resident f32
    tile (alternating HWDGE rings, sized so both rings carry ~equal queue
    time including the stores). Compute runs in letter chunks; relu'd
    outputs stream out on the scalar ring.
    """
    t_total = rows // P                  # letters per partition (224)
    if read_sizes is None:
        read_sizes = [4, 4, 6, 14, 14, 14, 28, 28, 28, 28, 28, 28]
    if chunk_sizes is None:
        chunk_sizes = [7, 7, 14, 28, 28, 28, 28, 28, 14, 14, 7, 7, 7, 7]
    assert sum(read_sizes) == t_total and sum(chunk_sizes) == t_total
    t_c_max = max(chunk_sizes)

    nc = bass.Bass(
        "TRN2", target_bir_lowering=False, debug=False, num_devices=N_CORES
    )
    x = nc.dram_tensor("x", [rows, L], F32, kind="ExternalInput")
    y = nc.dram_tensor("y", [rows, OUT], F32, kind="ExternalOutput")

    # partition p holds letters [p*t_total, (p+1)*t_total)
    xf = x.ap().rearrange("(p t) m -> p (t m)", p=P)   # [P, t_total*128]
    yf = y.ap().rearrange("(p t) m -> p (t m)", p=P)   # [P, t_total*84]

    with tile.TileContext(nc) as tc:
        with (
            tc.tile_pool(name="xin", bufs=1) as xin_pool,
            tc.tile_pool(name="xbc", bufs=xb_bufs) as xbc_pool,
            tc.tile_pool(name="rpool", bufs=work_bufs) as rpool,
            tc.tile_pool(name="spool", bufs=work_bufs) as spool,
            tc.tile_pool(name="upool", bufs=u_bufs) as upool,
            tc.tile_pool(name="oout", bufs=o_bufs) as oout_pool,
        ):
            xt = xin_pool.tile([P, t_total * L], F32, tag="x")
            # all reads upfront into disjoint slices -> max read-ahead
            off = 0
            for k, sz in enumerate(read_sizes):
                eng = nc.scalar if k % 2 == 1 else nc.sync
                eng.dma_start(
                    out=xt[:, off * L : (off + sz) * L],
                    in_=xf[:, off * L : (off + sz) * L],
                )
                off += sz

            # casts run one chunk ahead of the rest of the pipeline so the
            # in-order ACT engine never parks a ready cast behind a relu
            def alloc_xbc(t_c):
                return xbc_pool.tile([P, t_c_max * L], BF16, tag="xb", name="xb")[:, : t_c * L]

            def make_cast(ci, t_c, off):
                # ACT casts the first act_cast_letters; the DVE picks up the
                # remainder at the head of its per-chunk instruction group
                # (emitted in the main loop) to offload the pacing ACT engine.
                xbc = alloc_xbc(t_c)
                a = min(t_c, act_cast_letters) if ci >= pool_op2_min else t_c
                nc.scalar.activation(
                    xbc[:, : a * L], xt[:, off * L : (off + a) * L],
                    mybir.ActivationFunctionType.Copy,
                )
                return xbc, a

            offs = [sum(chunk_sizes[:i]) for i in range(len(chunk_sizes))]
            xbcs = {0: make_cast(0, chunk_sizes[0], 0)}
            off = 0
            for ci, t_c in enumerate(chunk_sizes):
                nci = ci + 1
                if nci < len(chunk_sizes):
                    xbcs[nci] = make_cast(nci, chunk_sizes[nci], offs[nci])
                xbc, a_done = xbcs.pop(ci)
                if a_done < t_c:
                    nc.vector.tensor_copy(
                        xbc[:, a_done * L : t_c * L],
                        xt[:, (off + a_done) * L : (off + t_c) * L],
                    )
                X3 = xbc.rearrange("p (row c) -> p row c", c=8)        # [P,t16,8]
                X4 = xbc.rearrange("p (t i j) -> p t i j", i=16, j=8)  # [P,t,16,8]

                # ---- op1: h(i,j) = x(i,j) + x(i,j+2)  (bf16, 2x) ----
                r = rpool.tile([P, t_c_max * 96], BF16, tag="r", name="r")[:, : t_c * 96]
                r3 = r.rearrange("p (row c) -> p row c", c=6)
                nc.vector.tensor_tensor(
                    r3[:], X3[:, :, 0:6], X3[:, :, 2:8], mybir.AluOpType.add
                )

                # ---- op2: s = h(0:14) + h(2:16) ----
                s = spool.tile([P, t_c_max * 84], BF16, tag="s", name="s")[:, : t_c * 84]
                r4 = r.rearrange("p (t i j) -> p t i j", i=16, j=6)
                s4 = s.rearrange("p (t i j) -> p t i j", i=14, j=6)
                op2eng = nc.gpsimd if ci >= pool_op2_min else nc.vector
                op2eng.tensor_tensor(
                    s4[:], r4[:, :, 0:14, :], r4[:, :, 2:16, :],
                    mybir.AluOpType.add,
                )

                # ---- op3: u = s + center taps ----
                u = upool.tile([P, t_c_max * 84], BF16, tag="u", name="u")[:, : t_c * 84]
                u4 = u.rearrange("p (t i j) -> p t i j", i=14, j=6)
                nc.vector.tensor_tensor(
                    u4[:], s4[:], X4[:, :, 1:15, 1:7], mybir.AluOpType.add
                )

                # ---- relu (bf16 -> f32) + store on the scalar ring ----
                ot = oout_pool.tile([P, t_c_max * OUT], F32, tag="o", name="ot")[:, : t_c * OUT]
                if ci in pool_relu_chunks:
                    nc.gpsimd.tensor_scalar_max(ot[:], u[:], 0.0)
                else:
                    nc.scalar.activation(
                        ot[:], u[:], mybir.ActivationFunctionType.Relu
                    )
                oring = nc.scalar if ci % 2 == 0 else nc.sync
                oring.dma_start(
                    out=yf[:, off * OUT : (off + t_c) * OUT], in_=ot[:]
                )
                off += t_c

    if split_waits:
        split_multi_waits(nc)
    return nc


_nc_cache = {}


def _get_program():
    if "nc" not in _nc_cache:
        _nc_cache["nc"] = build_program()
    return _nc_cache["nc"]


def kernel(x):
    x = np.ascontiguousarray(np.asarray(x, dtype=np.float32))
    assert x.shape == (B, W, L), x.shape

    nc = _get_program()
    shards = x.reshape(N_CORES, ROWS_PER_CORE, L)
    in_maps = [{"x": shards[i]} for i in range(N_CORES)]
    res = run_bass_kernel_spmd(nc, in_maps, core_ids=list(range(N_CORES)))
    out = np.concatenate(
        [res.results[i]["y"].reshape(-1, W, OUT) for i in range(N_CORES)], axis=0
    )
    return out


# revision 24
# speedup vs baseline: 1.1884x; 1.1884x over previous
"""Trainium2 Bass kernel for nn_Conv_27693949125154.

Each 128-dim vector is a 16x8 image; valid 3x3 conv with the fixed kernel
[[1,0,1],[0,1,0],[1,0,1]] then relu -> 84 outputs (14x6).

The conv kernel decomposes as outer([1,0,1],[1,0,1]) + center tap:
    h(i,j) = x(i,j) + x(i,j+2)            (horizontal, 16x6)
    out(i,j) = relu(h(i,j) + h(i+2,j) + x(i+1,j+1))   (14x6)

Layout: letters (B*W rows) on SBUF partitions, pixels along the free dim.
All stencil taps are free-dim strided slices.

Measured on HW: f32 DVE tensor-adds run 1 elem/cycle and stretch badly
under SBUF contention; bf16 adds hit the 2x DVE mode (0.52 ns/elem) with
no stretch; Pool dtype-converting ops are ~3.5 ns/elem (avoid). So: cast
each chunk f32->bf16 once on ACT, keep every DVE op bf16.

  whole input f32 -> resident SBUF tile, reads issued upfront on both
  HWDGE rings (deep read-ahead; input fully lands by ~45us).
  per 28-letter chunk:
    cast  (ACT): xbc = bf16(xt[chunk])     128/letter
    op1   (DVE): r = xbc(i,j)+xbc(i,j+2)   16x6/letter, 2x mode
    op2   (DVE): s = r(0:14)+r(2:16)       14x6
    op3   (DVE): u = s + xbc(i+1,j+1)      14x6
    relu  (ACT): ot = relu(u)              bf16 -> f32
    store (scalar ring)

Pure data parallel over 8 NeuronCores (batch sharding, no comm).
"""

import numpy as np

import concourse.bass as bass
import concourse.mybir as mybir
from concourse import tile
from concourse.bass_utils import run_bass_kernel_spmd

# Full problem: x (16384, 14, 128) f32 -> out (16384, 14, 84) f32
B, W, L = 16384, 14, 128
OUT = 84
N_CORES = 8
ROWS = B * W                     # 229376 letters total
ROWS_PER_CORE = ROWS // N_CORES  # 28672
P = 128                          # SBUF partitions

F32 = mybir.dt.float32
BF16 = mybir.dt.bfloat16


def split_multi_waits(nc, max_waits=1):
    """walrus CoreV3 codegen rejects instructions with several sync-wait
    conditions; hoist extras onto NOPs inserted just before, same engine."""
    for f in nc.m.functions:
        for blk in f.blocks:
            new = []
            for inst in blk.instructions:
                si = inst.sync_info
                if si is not None and si.on_wait and len(si.on_wait) > max_waits:
                    waits = list(si.on_wait)
                    head, tail = waits[:-max_waits], waits[-max_waits:]
                    for k, w in enumerate(head):
                        new.append(
                            mybir.InstNoOp(
                                name=f"{inst.name}-wsplit{k}",
                                engine=inst.engine,
                                ins=[],
                                outs=[],
                                sync_info=mybir.SyncInfo(on_wait=[w], on_update=[]),
                            )
                        )
                    inst.sync_info = mybir.SyncInfo(
                        on_wait=tail, on_update=list(si.on_update)
                    )
                new.append(inst)
            blk.instructions = new


def build_program(rows=ROWS_PER_CORE, read_sizes=None, chunk_sizes=None,
                  split_waits=True, xb_bufs=2, work_bufs=2, u_bufs=3, o_bufs=4,
                  pool_relu_chunks=(), dve_cast_chunks=(), pool_op2_min=99,
                  act_cast_letters=99):
    """Per-core program: x [rows,128] f32 -> y [rows,84] f32.

    Reads are issued upfront as independent slice-DMAs into a resident f32
    tile (alternating HWDGE rings, sized so both rings carry ~equal queue
    time including the stores). Compute runs in letter chunks; relu'd
    outputs stream out on the scalar ring.
    """
    t_total = rows // P                  # letters per partition (224)
    if read_sizes is None:
        read_sizes = [4, 4, 6, 14, 14, 14, 28, 28, 28, 28, 28, 28]
    if chunk_sizes is None:
        chunk_sizes = [7, 7, 14, 28, 28, 28, 28, 28, 14, 14, 7, 7, 7, 7]
    assert sum(read_sizes) == t_total and sum(chunk_sizes) == t_total
    t_c_max = max(chunk_sizes)

    nc = bass.Bass(
        "TRN2", target_bir_lowering=False, debug=False, num_devices=N_CORES
    )
    x = nc.dram_tensor("x", [rows, L], F32, kind="ExternalInput")
    y = nc.dram_tensor("y", [rows, OUT], F32, kind="ExternalOutput")

    # partition p holds letters [p*t_total, (p+1)*t_total)
    xf = x.ap().rearrange("(p t) m -> p (t m)", p=P)   # [P, t_total*128]
    yf = y.ap().rearrange("(p t) m -> p (t m)", p=P)   # [P, t_total*84]

    with tile.TileContext(nc) as tc:
        with (
            tc.tile_pool(name="xin", bufs=1) as xin_pool,
            tc.tile_pool(name="xbc", bufs=xb_bufs) as xbc_pool,
            tc.tile_pool(name="rpool", bufs=work_bufs) as rpool,
            tc.tile_pool(name="spool", bufs=work_bufs) as spool,
            tc.tile_pool(name="upool", bufs=u_bufs) as upool,
            tc.tile_pool(name="oout", bufs=o_bufs) as oout_pool,
        ):
            xt = xin_pool.tile([P, t_total * L], F32, tag="x")
            # all reads upfront into disjoint slices -> max read-ahead
            off = 0
            for k, sz in enumerate(read_sizes):
                eng = nc.scalar if (k % 2 == 1 and k < 8) else nc.sync
                eng.dma_start(
                    out=xt[:, off * L : (off + sz) * L],
                    in_=xf[:, off * L : (off + sz) * L],
                )
                off += sz

            # casts run one chunk ahead of the rest of the pipeline so the
            # in-order ACT engine never parks a ready cast behind a relu
            def alloc_xbc(t_c):
                return xbc_pool.tile([P, t_c_max * L], BF16, tag="xb", name="xb")[:, : t_c * L]

            def make_cast(ci, t_c, off):
                # ACT casts the first act_cast_letters; the DVE picks up the
                # remainder at the head of its per-chunk instruction group
                # (emitted in the main loop) to offload the pacing ACT engine.
                xbc = alloc_xbc(t_c)
                a = min(t_c, act_cast_letters) if ci >= pool_op2_min else t_c
                nc.scalar.activation(
                    xbc[:, : a * L], xt[:, off * L : (off + a) * L],
                    mybir.ActivationFunctionType.Copy,
                )
                return xbc, a

            offs = [sum(chunk_sizes[:i]) for i in range(len(chunk_sizes))]
            xbcs = {0: make_cast(0, chunk_sizes[0], 0)}
            off = 0
            for ci, t_c in enumerate(chunk_sizes):
                nci = ci + 1
                if nci < len(chunk_sizes):
                    xbcs[nci] = make_cast(nci, chunk_sizes[nci], offs[nci])
                xbc, a_done = xbcs.pop(ci)
                if a_done < t_c:
                    nc.vector.tensor_copy(
                        xbc[:, a_done * L : t_c * L],
                        xt[:, (off + a_done) * L : (off + t_c) * L],
                    )
                X3 = xbc.rearrange("p (row c) -> p row c", c=8)        # [P,t16,8]
                X4 = xbc.rearrange("p (t i j) -> p t i j", i=16, j=8)  # [P,t,16,8]

                # ---- op1: h(i,j) = x(i,j) + x(i,j+2)  (bf16, 2x) ----
                r = rpool.tile([P, t_c_max * 96], BF16, tag="r", name="r")[:, : t_c * 96]
                r3 = r.rearrange("p (row c) -> p row c", c=6)
                nc.vector.tensor_tensor(
                    r3[:], X3[:, :, 0:6], X3[:, :, 2:8], mybir.AluOpType.add
                )

                # ---- op2: s = h(0:14) + h(2:16) ----
                s = spool.tile([P, t_c_max * 84], BF16, tag="s", name="s")[:, : t_c * 84]
                r4 = r.rearrange("p (t i j) -> p t i j", i=16, j=6)
                s4 = s.rearrange("p (t i j) -> p t i j", i=14, j=6)
                op2eng = nc.gpsimd if ci >= pool_op2_min else nc.vector
                op2eng.tensor_tensor(
                    s4[:], r4[:, :, 0:14, :], r4[:, :, 2:16, :],
                    mybir.AluOpType.add,
                )

                # ---- op3: u = s + center taps ----
                u = upool.tile([P, t_c_max * 84], BF16, tag="u", name="u")[:, : t_c * 84]
                u4 = u.rearrange("p (t i j) -> p t i j", i=14, j=6)
                nc.vector.tensor_tensor(
                    u4[:], s4[:], X4[:, :, 1:15, 1:7], mybir.AluOpType.add
                )

                # ---- relu (bf16 -> f32) + store on the scalar ring ----
                ot = oout_pool.tile([P, t_c_max * OUT], F32, tag="o", name="ot")[:, : t_c * OUT]
                if ci in pool_relu_chunks:
                    nc.gpsimd.tensor_scalar_max(ot[:], u[:], 0.0)
                else:
                    nc.scalar.activation(
                        ot[:], u[:], mybir.ActivationFunctionType.Relu
                    )
                oring = nc.scalar if ci % 2 == 0 else nc.sync
                oring.dma_start(
                    out=yf[:, off * OUT : (off + t_c) * OUT], in_=ot[:]
                )
                off += t_c

    if split_waits:
        split_multi_waits(nc)
    return nc


_nc_cache = {}


def _get_program():
    if "nc" not in _nc_cache:
        _nc_cache["nc"] = build_program()
    return _nc_cache["nc"]


def kernel(x):
    x = np.ascontiguousarray(np.asarray(x, dtype=np.float32))
    assert x.shape == (B, W, L), x.shape

    nc = _get_program()
    shards = x.reshape(N_CORES, ROWS_PER_CORE, L)
    in_maps = [{"x": shards[i]} for i in range(N_CORES)]
    res = run_bass_kernel_spmd(nc, in_maps, core_ids=list(range(N_CORES)))
    out = np.concatenate(
        [res.results[i]["y"].reshape(-1, W, OUT) for i in range(N_CORES)], axis=0
    )
    return out


# revision 25
# speedup vs baseline: 1.2557x; 1.0566x over previous
"""Trainium2 Bass kernel for nn_Conv_27693949125154.

Each 128-dim vector is a 16x8 image; valid 3x3 conv with the fixed kernel
[[1,0,1],[0,1,0],[1,0,1]] then relu -> 84 outputs (14x6).

The conv kernel decomposes as outer([1,0,1],[1,0,1]) + center tap:
    h(i,j) = x(i,j) + x(i,j+2)            (horizontal, 16x6)
    out(i,j) = relu(h(i,j) + h(i+2,j) + x(i+1,j+1))   (14x6)

Layout: letters (B*W rows) on SBUF partitions, pixels along the free dim.
All stencil taps are free-dim strided slices.

Measured on HW: f32 DVE tensor-adds run 1 elem/cycle and stretch badly
under SBUF contention; bf16 adds hit the 2x DVE mode (0.52 ns/elem) with
no stretch; Pool dtype-converting ops are ~3.5 ns/elem (avoid). So: cast
each chunk f32->bf16 once on ACT, keep every DVE op bf16.

  whole input f32 -> resident SBUF tile, reads issued upfront on both
  HWDGE rings, most on sync (the store-carrying scalar ring must keep a
  short read backlog or stores park behind it). Chunks taper small at both
  ends (fast ramp, short tail). Deep ot/u pools (o=4/u=3) decouple compute
  from store-DMA completion - without them the pipeline stalls ~12us.
  per chunk (28 letters mid-stream, 7-14 at the edges):
    cast  (ACT): xbc = bf16(xt[chunk]), issued one chunk ahead
    op1   (DVE): r = xbc(i,j)+xbc(i,j+2)   16x6/letter, 2x mode
    op2   (DVE): s = r(0:14)+r(2:16)       14x6
    op3   (DVE): u = s + xbc(i+1,j+1)      14x6
    relu  (ACT): ot = relu(u)              bf16 -> f32
    store (rings alternate per chunk)

Pure data parallel over 8 NeuronCores (batch sharding, no comm).
"""

import numpy as np

import concourse.bass as bass
import concourse.mybir as mybir
from concourse import tile
from concourse.bass_utils import run_bass_kernel_spmd

# Full problem: x (16384, 14, 128) f32 -> out (16384, 14, 84) f32
B, W, L = 16384, 14, 128
OUT = 84
N_CORES = 8
ROWS = B * W                     # 229376 letters total
ROWS_PER_CORE = ROWS // N_CORES  # 28672
P = 128                          # SBUF partitions

F32 = mybir.dt.float32
BF16 = mybir.dt.bfloat16


def split_multi_waits(nc, max_waits=1):
    """walrus CoreV3 codegen rejects instructions with several sync-wait
    conditions; hoist extras onto NOPs inserted just before, same engine."""
    for f in nc.m.functions:
        for blk in f.blocks:
            new = []
            for inst in blk.instructions:
                si = inst.sync_info
                if si is not None and si.on_wait and len(si.on_wait) > max_waits:
                    waits = list(si.on_wait)
                    head, tail = waits[:-max_waits], waits[-max_waits:]
                    for k, w in enumerate(head):
                        new.append(
                            mybir.InstNoOp(
                                name=f"{inst.name}-wsplit{k}",
                                engine=inst.engine,
                                ins=[],
                                outs=[],
                                sync_info=mybir.SyncInfo(on_wait=[w], on_update=[]),
                            )
                        )
                    inst.sync_info = mybir.SyncInfo(
                        on_wait=tail, on_update=list(si.on_update)
                    )
                new.append(inst)
            blk.instructions = new


def build_program(rows=ROWS_PER_CORE, read_sizes=None, chunk_sizes=None,
                  split_waits=True, xb_bufs=2, work_bufs=2, u_bufs=3, o_bufs=4,
                  pool_relu_chunks=(), dve_cast_chunks=(), pool_op2_min=99,
                  act_cast_letters=99):
    """Per-core program: x [rows,128] f32 -> y [rows,84] f32.

    Reads are issued upfront as independent slice-DMAs into a resident f32
    tile (alternating HWDGE rings, sized so both rings carry ~equal queue
    time including the stores). Compute runs in letter chunks; relu'd
    outputs stream out on the scalar ring.
    """
    t_total = rows // P                  # letters per partition (224)
    if read_sizes is None:
        read_sizes = [4, 4, 6, 14, 14, 14, 28, 28, 28, 28, 28, 28]
    if chunk_sizes is None:
        chunk_sizes = [7, 7, 14, 28, 28, 28, 28, 28, 14, 14, 7, 7, 7, 7]
    assert sum(read_sizes) == t_total and sum(chunk_sizes) == t_total
    t_c_max = max(chunk_sizes)

    nc = bass.Bass(
        "TRN2", target_bir_lowering=False, debug=False, num_devices=N_CORES
    )
    x = nc.dram_tensor("x", [rows, L], F32, kind="ExternalInput")
    y = nc.dram_tensor("y", [rows, OUT], F32, kind="ExternalOutput")

    # partition p holds letters [p*t_total, (p+1)*t_total)
    xf = x.ap().rearrange("(p t) m -> p (t m)", p=P)   # [P, t_total*128]
    yf = y.ap().rearrange("(p t) m -> p (t m)", p=P)   # [P, t_total*84]

    with tile.TileContext(nc) as tc:
        with (
            tc.tile_pool(name="xin", bufs=1) as xin_pool,
            tc.tile_pool(name="xbc", bufs=xb_bufs) as xbc_pool,
            tc.tile_pool(name="rpool", bufs=work_bufs) as rpool,
            tc.tile_pool(name="spool", bufs=work_bufs) as spool,
            tc.tile_pool(name="upool", bufs=u_bufs) as upool,
            tc.tile_pool(name="oout", bufs=o_bufs) as oout_pool,
        ):
            xt = xin_pool.tile([P, t_total * L], F32, tag="x")
            # all reads upfront into disjoint slices -> max read-ahead
            off = 0
            for k, sz in enumerate(read_sizes):
                eng = nc.scalar if (k % 2 == 1 and k < 8) else nc.sync
                eng.dma_start(
                    out=xt[:, off * L : (off + sz) * L],
                    in_=xf[:, off * L : (off + sz) * L],
                )
                off += sz

            # casts run one chunk ahead of the rest of the pipeline so the
            # in-order ACT engine never parks a ready cast behind a relu
            def alloc_xbc(t_c):
                return xbc_pool.tile([P, t_c_max * L], BF16, tag="xb", name="xb")[:, : t_c * L]

            def make_cast(ci, t_c, off):
                # ACT casts the first act_cast_letters; the DVE picks up the
                # remainder at the head of its per-chunk instruction group
                # (emitted in the main loop) to offload the pacing ACT engine.
                xbc = alloc_xbc(t_c)
                a = min(t_c, act_cast_letters) if ci >= pool_op2_min else t_c
                nc.scalar.activation(
                    xbc[:, : a * L], xt[:, off * L : (off + a) * L],
                    mybir.ActivationFunctionType.Copy,
                )
                return xbc, a

            offs = [sum(chunk_sizes[:i]) for i in range(len(chunk_sizes))]
            xbcs = {0: make_cast(0, chunk_sizes[0], 0)}
            off = 0
            for ci, t_c in enumerate(chunk_sizes):
                nci = ci + 1
                if nci < len(chunk_sizes):
                    xbcs[nci] = make_cast(nci, chunk_sizes[nci], offs[nci])
                xbc, a_done = xbcs.pop(ci)
                if a_done < t_c:
                    nc.vector.tensor_copy(
                        xbc[:, a_done * L : t_c * L],
                        xt[:, (off + a_done) * L : (off + t_c) * L],
                    )
                X3 = xbc.rearrange("p (row c) -> p row c", c=8)        # [P,t16,8]
                X4 = xbc.rearrange("p (t i j) -> p t i j", i=16, j=8)  # [P,t,16,8]

                # ---- op1: h(i,j) = x(i,j) + x(i,j+2)  (bf16, 2x) ----
                r = rpool.tile([P, t_c_max * 96], BF16, tag="r", name="r")[:, : t_c * 96]
                r3 = r.rearrange("p (row c) -> p row c", c=6)
                nc.vector.tensor_tensor(
                    r3[:], X3[:, :, 0:6], X3[:, :, 2:8], mybir.AluOpType.add
                )

                # ---- op2: s = h(0:14) + h(2:16) ----
                s = spool.tile([P, t_c_max * 84], BF16, tag="s", name="s")[:, : t_c * 84]
                r4 = r.rearrange("p (t i j) -> p t i j", i=16, j=6)
                s4 = s.rearrange("p (t i j) -> p t i j", i=14, j=6)
                op2eng = nc.gpsimd if ci >= pool_op2_min else nc.vector
                op2eng.tensor_tensor(
                    s4[:], r4[:, :, 0:14, :], r4[:, :, 2:16, :],
                    mybir.AluOpType.add,
                )

                # ---- op3: u = s + center taps ----
                u = upool.tile([P, t_c_max * 84], BF16, tag="u", name="u")[:, : t_c * 84]
                u4 = u.rearrange("p (t i j) -> p t i j", i=14, j=6)
                nc.vector.tensor_tensor(
                    u4[:], s4[:], X4[:, :, 1:15, 1:7], mybir.AluOpType.add
                )

                # ---- relu (bf16 -> f32) + store on the scalar ring ----
                ot = oout_pool.tile([P, t_c_max * OUT], F32, tag="o", name="ot")[:, : t_c * OUT]
                if ci in pool_relu_chunks:
                    nc.gpsimd.tensor_scalar_max(ot[:], u[:], 0.0)
                else:
                    nc.scalar.activation(
                        ot[:], u[:], mybir.ActivationFunctionType.Relu
                    )
                oring = nc.scalar if ci % 2 == 0 else nc.sync
                oring.dma_start(
                    out=yf[:, off * OUT : (off + t_c) * OUT], in_=ot[:]
                )
                off += t_c

    if split_waits:
        split_multi_waits(nc)
    return nc


_nc_cache = {}


def _get_program():
    if "nc" not in _nc_cache:
        _nc_cache["nc"] = build_program()
    return _nc_cache["nc"]


def kernel(x):
    x = np.ascontiguousarray(np.asarray(x, dtype=np.float32))
    assert x.shape == (B, W, L), x.shape

    nc = _get_program()
    shards = x.reshape(N_CORES, ROWS_PER_CORE, L)
    in_maps = [{"x": shards[i]} for i in range(N_CORES)]
    res = run_bass_kernel_spmd(nc, in_maps, core_ids=list(range(N_CORES)))
    out = np.concatenate(
        [res.results[i]["y"].reshape(-1, W, OUT) for i in range(N_CORES)], axis=0
    )
    return out
